# revision 55
# baseline (speedup 1.0000x reference)
"""Trainium2 Bass kernel for 2-layer GAT (nn_GAT_5970004541990).

Sharding: nodes padded 50000 -> 50176, sharded by destination across 8
NeuronCores (6272 nodes = 49 tiles of 128 each). Self-loops added. Host does
index preprocessing only; all FLOPs run on device.

v2 schedule: per dst tile, most edges sit in "dst-major" chunks where slot p
holds the r-th edge of dst node p, so the attention one-hot is the identity
and ald comes from a per-tile column (one 6272-row gather per layer instead
of one 256B row per edge). High-degree tails go to packed "overflow" chunks
that keep the one-hot path with a per-edge ald gather. Invalid dst-major
slots gather a dedicated all-zero poison row (contributes nothing).

Per core:
- Phase 1 (replicated): node table h_aug = x @ [W1 | W1 a1s | W1 a1d] + ones
  column in bf16, written to DRAM split at row 32640 (signed int16 gather
  indices); poison row appended to each split. ald column copied to a
  pair-packed side table.
- Phase 2 (layer 1): per super-tile of 2 dst tiles, dma_gather edge-chunk
  source rows; e = als[src] + ald[dst]; p = exp(leaky_relu(e)); per chunk a
  PE matmul U += Mp^T @ [feat|...|ones] accumulates numerator + denominator
  in PSUM (Mp = p-scaled identity for dst-major chunks, p-scaled one-hot for
  overflow chunks, built on the Activation engine); out1 = relu(U/s);
  layer-2 table shard h2_aug = out1 @ [W2 | W2 a2s | W2 a2d].
- Phase 3: AllGather compact layer-2 shards (50176 x 19 f32), expand into
  256B-row tables (+ poison rows).
- Phase 4 (layer 2): same aggregation, no relu; per-tile pooling matmul
  accumulates [64, 16] graph sums in PSUM.
- Phase 5: AllReduce pooled sums; mean via host 1/count; log_softmax.
All cores emit the identical [64, 16] output; kernel() returns core 0's.
"""
import sys

sys.path.insert(0, "/opt/trn_rl_repo")

import numpy as np

import concourse.bass as bass
import concourse.mybir as mybir
import concourse.tile as tile
from concourse.tile_rust import add_dep_helper
from concourse import bacc
from concourse.bass_utils import run_bass_kernel_spmd
from concourse.masks import make_identity

# ---------------------------------------------------------------- constants
N_NODES = 50000
N_GRAPHS = 64
D_HID = 128
D_OUT = 16
NEG = 0.2

P = 128
NCORES = 8
NPAD = 50176                     # 8 * 49 * 128
NODES_PER_CORE = NPAD // NCORES  # 6272
TILES_PER_CORE = NODES_PER_CORE // P  # 49
GLOBAL_TILES = NPAD // P         # 392
NLO = 32640                      # table split (signed int16 index limit,
NHI = NPAD - NLO                 # minus room for one poison row each)
LO_TILES = NLO // P              # 255
NAD = NPAD // 2                  # 25088 ald-table rows (pair-packed, < 32768)

S_TILES = 2                      # dst tiles per super-tile
MP_BATCH = 4                     # chunks per batched Mp build on DVE
G_BUFS = 4                       # aggregation pool depth
U2_BUFS = 4                      # layer-2 U psum bufs
MP_BUFS = 8
XB_P1 = 8                        # phase-1 tiles per batched DMA
P1PS_BUFS = 4
U1_BUFS = 4
ELEM1B = 256                     # layer-1 table row bf16 (512B)
ELEM2 = 64                       # layer-2 table row f32 (256B)
ELEM_AD1 = 128                   # ald side table row bf16 (256B)
ROW1 = 131                       # meaningful cols: feat 0..127, als, ald, ones
ROW2 = 19                        # feat 0..15, als2, ald2, ones

f32 = mybir.dt.float32
i16 = mybir.dt.int16
bf16 = mybir.dt.bfloat16


def _ceil(a, b):
    return -(-a // b)


def _wrap_idx(flat):
    """dma_gather index layout: slot i reads wrapped[i%16, i//16]; the 16-row
    pattern is replicated to all 128 partitions (one per Q7 core)."""
    v16 = flat.astype(np.uint16).view(np.int16)
    return np.tile(v16.reshape(-1, 16).T, (8, 1))


def _pad_to(a, n, fill):
    out = np.full(n, fill, a.dtype)
    out[:len(a)] = a
    return out


# ------------------------------------------------------------ host indexing
def preprocess(edge_index, batch):
    src = np.concatenate(
        [edge_index[0].astype(np.int64), np.arange(NPAD, dtype=np.int64)])
    dst = np.concatenate(
        [edge_index[1].astype(np.int64), np.arange(NPAD, dtype=np.int64)])
    core = dst // NODES_PER_CORE
    edge_hi = src >= NLO

    # per-core per-(tile, lo/hi) edge counts -> global ceil chunk counts
    nseg = np.zeros((2, NCORES, TILES_PER_CORE), np.int64)
    seg_of_edge = (core * TILES_PER_CORE
                   + (dst % NODES_PER_CORE) // P)
    np.add.at(nseg[0].reshape(-1), seg_of_edge[~edge_hi], 1)
    np.add.at(nseg[1].reshape(-1), seg_of_edge[edge_hi], 1)
    CN = _ceil(nseg, P).max(axis=1)      # [2, T] chunks per (h, tile)

    # static chunk schedule (identical on every core); per super the lo
    # chunks come first so each of the two gathers covers one chunk range
    supers = []
    chunk_tiles, chunk_start, chunk_stop = [], [], []
    lo_c = hi_c = 0
    for s0 in range(0, TILES_PER_CORE, S_TILES):
        tiles = list(range(s0, min(s0 + S_TILES, TILES_PER_CORE)))
        chunks = [(t, 0) for t in tiles for _ in range(int(CN[0, t]))]
        n_lo = len(chunks)
        chunks += [(t, 1) for t in tiles for _ in range(int(CN[1, t]))]
        n_hi = len(chunks) - n_lo
        first, last = {}, {}
        for ci, (t, _) in enumerate(chunks):
            first.setdefault(t, ci)
            last[t] = ci
        base = len(chunk_tiles)
        supers.append(dict(tiles=tiles, chunks=chunks,
                           n_lo=n_lo, n_hi=n_hi, base=base,
                           lo_col0=lo_c, hi_col0=hi_c))
        lo_c += n_lo * 8
        hi_c += n_hi * 8
        for ci, (t, _) in enumerate(chunks):
            chunk_tiles.append(t)
            chunk_start.append(ci == first[t])
            chunk_stop.append(ci == last[t])
    TC = len(chunk_tiles)

    # per-core index tensors
    batch_pad = np.full(NPAD, 127, np.int64)
    batch_pad[:N_NODES] = batch.astype(np.int64)
    per_core = []
    for k in range(NCORES):
        m = core == k
        s_k = src[m]
        d_k = dst[m] - k * NODES_PER_CORE
        h_k = edge_hi[m]
        t_k = d_k // P
        o_k = d_k % P
        lists = {}
        for t in range(TILES_PER_CORE):
            for h in (0, 1):
                mm = (t_k == t) & (h_k == (h == 1))
                s = s_k[mm] - (NLO if h else 0)
                o = o_k[mm]
                order = np.argsort(o, kind="stable")
                lists[(h, t)] = (s[order], o[order])

        lo_cols, hi_cols = [], []
        dstoff = np.full((TC, P), -1, np.int16)
        for s in supers:
            lo_flat, hi_flat = [], []
            pos = {}
            for ci, (t, h) in enumerate(s["chunks"]):
                gc = s["base"] + ci
                r = pos.get((h, t), 0)
                pos[(h, t)] = r + 1
                srcs, offs = lists[(h, t)]
                nch = int(CN[h, t])
                flat = _pad_to(srcs, nch * P, 0)[r * P:(r + 1) * P]
                off2 = _pad_to(offs.astype(np.int16), nch * P,
                               np.int16(-1))[r * P:(r + 1) * P]
                dstoff[gc] = off2
                (lo_flat if h == 0 else hi_flat).append(flat)
            lo_cols.append(_wrap_idx(np.concatenate(lo_flat)))
            if hi_flat:
                hi_cols.append(_wrap_idx(np.concatenate(hi_flat)))
        idx_lo = np.concatenate(lo_cols, axis=1)
        idx_hi = (np.concatenate(hi_cols, axis=1) if hi_cols
                  else np.zeros((P, 1), np.int16))
        gn = k * NODES_PER_CORE + np.arange(NODES_PER_CORE, dtype=np.int64)
        boff = batch_pad[k * NODES_PER_CORE:(k + 1) * NODES_PER_CORE]
        per_core.append(dict(
            idx_lo=np.ascontiguousarray(idx_lo),
            idx_hi=np.ascontiguousarray(idx_hi),
            dstoff=np.ascontiguousarray(dstoff.T),
            idx_adall=np.ascontiguousarray(_wrap_idx(gn % NAD)),
            admall=np.ascontiguousarray(
                (gn >= NAD).astype(np.float32).reshape(TILES_PER_CORE, P).T),
            batchoff=np.ascontiguousarray(
                boff.reshape(TILES_PER_CORE, P).T.astype(np.int16))))

    meta = dict(supers=supers, TC=TC,
                chunk_start=chunk_start, chunk_stop=chunk_stop,
                idx_lo_cols=per_core[0]["idx_lo"].shape[1],
                idx_hi_cols=per_core[0]["idx_hi"].shape[1])
    return meta, per_core


# ------------------------------------------------------------- bass program
def build_program(meta, with_bias1, with_bias2, repeats=None, ablate=(),
                  nqueues=4):
    nc = bacc.Bacc("TRN2", target_bir_lowering=False, debug=False,
                   num_devices=NCORES, num_swdge_queues=nqueues)
    rep = dict(p1=1, l1=1, l2=1)
    rep.update(repeats or {})
    t1_dt = bf16
    elem1 = ELEM1B
    TC = meta["TC"]
    supers = meta["supers"]
    AF = mybir.ActivationFunctionType
    OP = mybir.AluOpType
    core_ids = list(range(NCORES))

    xT_d = nc.dram_tensor("xT", [P, NPAD], bf16, kind="ExternalInput")
    w1_d = nc.dram_tensor("w1aug", [P, 130], bf16, kind="ExternalInput")
    w2_d = nc.dram_tensor("w2aug", [P, 18], f32, kind="ExternalInput")
    ilo_d = nc.dram_tensor("idx_lo", [P, meta["idx_lo_cols"]], i16,
                           kind="ExternalInput")
    ihi_d = nc.dram_tensor("idx_hi", [P, meta["idx_hi_cols"]], i16,
                           kind="ExternalInput")
    doff_d = nc.dram_tensor("dstoff", [P, TC], i16, kind="ExternalInput")
    iadall_d = nc.dram_tensor("idx_adall", [P, TILES_PER_CORE * 8], i16,
                              kind="ExternalInput")
    admall_d = nc.dram_tensor("admall", [P, TILES_PER_CORE], f32,
                              kind="ExternalInput")
    boff_d = nc.dram_tensor("batchoff", [P, TILES_PER_CORE], i16,
                            kind="ExternalInput")
    iota_d = nc.dram_tensor("iota", [P, P], i16, kind="ExternalInput")
    rcnt_d = nc.dram_tensor("recip_cnt", [N_GRAPHS, 1], f32,
                            kind="ExternalInput")
    if with_bias1:
        b1_d = nc.dram_tensor("b1b", [P, D_HID], f32, kind="ExternalInput")
    if with_bias2:
        b2_d = nc.dram_tensor("b2b", [P, D_OUT], f32, kind="ExternalInput")
    out_d = nc.dram_tensor("out", [N_GRAPHS, D_OUT], f32,
                           kind="ExternalOutput")

    # +1 row on each split: the all-zero poison row for invalid dm slots
    t1lo = nc.dram_tensor("t1lo", [NLO + 1, elem1], t1_dt)
    t1hi = nc.dram_tensor("t1hi", [NHI + 1, elem1], t1_dt)
    t1ad = nc.dram_tensor("t1ad", [NAD, ELEM_AD1], t1_dt)
    t2ad = nc.dram_tensor("t2ad", [NAD, ELEM2], f32)
    ag_in = nc.dram_tensor("ag_in", [NODES_PER_CORE, ROW2], f32)
    ag_out = nc.dram_tensor("ag_out", [NPAD, ROW2], f32, addr_space="Shared")
    t2lo = nc.dram_tensor("t2lo", [NLO + 1, ELEM2], f32)
    t2hi = nc.dram_tensor("t2hi", [NHI + 1, ELEM2], f32)
    ar_in = nc.dram_tensor("ar_in", [N_GRAPHS, D_OUT], f32)
    ar_out = nc.dram_tensor("ar_out", [N_GRAPHS, D_OUT], f32,
                            addr_space="Shared")

    def dep(after, *before):
        for b in before:
            add_dep_helper(after.ins, b.ins, reason="phase order")

    with tile.TileContext(nc) as tc:
        with tc.tile_pool(name="res", bufs=1) as res:
            iota_t = res.tile([P, P], i16)
            nc.sync.dma_start(out=iota_t[:], in_=iota_d.ap())
            doff_t = res.tile([P, TC], i16)
            nc.sync.dma_start(out=doff_t[:], in_=doff_d.ap())
            iadall_t = res.tile([P, TILES_PER_CORE * 8], i16)
            nc.sync.dma_start(out=iadall_t[:], in_=iadall_d.ap())
            admall_t = res.tile([P, TILES_PER_CORE], f32)
            nc.sync.dma_start(out=admall_t[:], in_=admall_d.ap())
            ilo_t = res.tile([P, meta["idx_lo_cols"]], i16)
            nc.sync.dma_start(out=ilo_t[:], in_=ilo_d.ap())
            ihi_t = res.tile([P, meta["idx_hi_cols"]], i16)
            nc.sync.dma_start(out=ihi_t[:], in_=ihi_d.ap())
            boff_t = res.tile([P, TILES_PER_CORE], i16)
            nc.sync.dma_start(out=boff_t[:], in_=boff_d.ap())
            w1_t = res.tile([P, 130], bf16)
            nc.sync.dma_start(out=w1_t[:], in_=w1_d.ap())
            w2_t = res.tile([P, 18], f32)
            nc.sync.dma_start(out=w2_t[:], in_=w2_d.ap())
            rcnt_t = res.tile([N_GRAPHS, 1], f32)
            nc.sync.dma_start(out=rcnt_t[:], in_=rcnt_d.ap())
            ident_t = res.tile([P, P], f32)
            make_identity(nc, ident_t[:])
            ones_row = res.tile([1, P], f32)
            nc.vector.memset(ones_row[:], 1.0)
            if with_bias1:
                b1_t = res.tile([P, D_HID], f32)
                nc.sync.dma_start(out=b1_t[:], in_=b1_d.ap())
            if with_bias2:
                b2_t = res.tile([P, D_OUT], f32)
                nc.sync.dma_start(out=b2_t[:], in_=b2_d.ap())

            # ---------------- phase 1: node table (replicated) -------------
            tab_writes = []
            lo_writes = []
            XB = XB_P1
            with (
                nc.named_scope("phase1_table"),
                tc.tile_pool(name="p1", bufs=3) as p1,
                tc.tile_pool(name="p1ps", bufs=P1PS_BUFS, space="PSUM") as p1ps,
            ):
              for _r in range(rep["p1"]):
                for gb in range(0, GLOBAL_TILES, XB):
                    nb = min(XB, GLOBAL_TILES - gb)
                    xt = p1.tile([P, XB * P], bf16, tag="xt")
                    nc.sync.dma_start(
                        out=xt[:, 0:nb * P],
                        in_=xT_d.ap()[:, gb * P:(gb + nb) * P])
                    stg = p1.tile([P, XB, ROW1], t1_dt, tag="stg")
                    for j in range(nb):
                        gt = gb + j
                        hps = p1ps.tile([P, 130], f32, tag="hps")
                        nc.tensor.matmul(hps[:], xt[:, j * P:(j + 1) * P],
                                         w1_t[:], start=True, stop=True)
                        nc.vector.tensor_copy(stg[:, j, 0:130], hps[:])
                    nc.vector.memset(stg[:, 0:nb, 130:131], 1.0)
                    # one DMA for nb tiles: DRAM rows (j*128+p) <-> src (p, j)
                    if gb + nb <= LO_TILES:
                        dst = t1lo.ap()[gb * P:(gb + nb) * P, 0:ROW1]
                        dst = dst.rearrange("(j p) c -> p j c", p=P)
                        w = nc.sync.dma_start(out=dst, in_=stg[:, 0:nb, :])
                        tab_writes.append(w)
                        lo_writes.append(w)
                    elif gb >= LO_TILES:
                        r0 = (gb - LO_TILES) * P
                        dst = t1hi.ap()[r0:r0 + nb * P, 0:ROW1]
                        dst = dst.rearrange("(j p) c -> p j c", p=P)
                        tab_writes.append(
                            nc.sync.dma_start(out=dst, in_=stg[:, 0:nb, :]))
                    else:
                        nlo_t = LO_TILES - gb
                        dst = t1lo.ap()[gb * P:NLO, 0:ROW1]
                        dst = dst.rearrange("(j p) c -> p j c", p=P)
                        w = nc.sync.dma_start(out=dst, in_=stg[:, 0:nlo_t, :])
                        tab_writes.append(w)
                        lo_writes.append(w)
                        dst = t1hi.ap()[0:(nb - nlo_t) * P, 0:ROW1]
                        dst = dst.rearrange("(j p) c -> p j c", p=P)
                        tab_writes.append(nc.sync.dma_start(
                            out=dst, in_=stg[:, nlo_t:nb, :]))
                # ald column table from the fat tables (3 strided copies,
                # ordered after every table write)
                fence0 = nc.sync.nop(nofuse=True, hint="fence_p1w")
                dep(fence0, *tab_writes)
                with nc.allow_non_contiguous_dma(
                        reason="2B/row ald column build"):
                    c129 = 129
                    for ad_dma in (
                        nc.sync.dma_start(
                            out=t1ad.ap()[:, 0:1],
                            in_=t1lo.ap()[0:NAD, c129:c129 + 1]),
                        nc.sync.dma_start(
                            out=t1ad.ap()[0:NLO - NAD, 1:2],
                            in_=t1lo.ap()[NAD:NLO, c129:c129 + 1]),
                        nc.sync.dma_start(
                            out=t1ad.ap()[NLO - NAD:NAD, 1:2],
                            in_=t1hi.ap()[0:NHI, c129:c129 + 1]),
                    ):
                        dep(ad_dma, fence0)
                        tab_writes.append(ad_dma)

            fence1 = nc.sync.nop(nofuse=True, hint="fence_p1")
            dep(fence1, *tab_writes)   # tab_writes includes the 3 ad builds

            # --------------- shared aggregation loop ----------------------
            def aggregation_layer(layer, pool, psum_u, misc):
                """misc: dict with layer-specific psum pools / tiles."""
                if layer == 1:
                    elem, row, als_c, ones_c = elem1, ROW1, 128, 130
                    tlo, thi, tad = t1lo, t1hi, t1ad
                    g_dt = t1_dt
                    elem_ad, ad_dt = ELEM_AD1, t1_dt
                else:
                    elem, row, als_c, ones_c = ELEM2, ROW2, 16, 18
                    tlo, thi, tad = t2lo, t2hi, t2ad
                    g_dt = f32
                    elem_ad, ad_dt = ELEM2, f32
                gathers = []
                ad_gathers = []
                side_writes = []
                qctr = [0]

                def nextq():
                    q = qctr[0] % nqueues
                    qctr[0] += 1
                    return q

                # per-layer ald column for this core's own nodes:
                # ala[p, t] = ald[core*6272 + t*128 + p]
                ADA = misc["ala"].tile([P, TILES_PER_CORE, elem_ad], ad_dt,
                                       tag="ADA")
                if "gathers" in ablate:
                    nc.vector.memset(ADA[:, :, 0:2], 0.5)
                else:
                    ga = nc.gpsimd.dma_gather(
                        out_ap=ADA[:], in_ap=tad.ap(),
                        idxs_ap=iadall_t[:],
                        num_idxs=NODES_PER_CORE, num_idxs_reg=NODES_PER_CORE,
                        elem_size=elem_ad, single_packet=False,
                        queue_num=nextq())
                    ad_gathers.append(ga)
                ala = misc["ala"].tile([P, TILES_PER_CORE], f32, tag="ala")
                nc.vector.tensor_sub(ala[:], ADA[:, :, 1], ADA[:, :, 0])
                nc.vector.tensor_mul(ala[:], ala[:], admall_t[:])
                nc.vector.tensor_add(ala[:], ala[:], ADA[:, :, 0])

                for si, s in enumerate(supers):
                    sc = len(s["chunks"])
                    n_lo, n_hi, base = s["n_lo"], s["n_hi"], s["base"]
                    G = pool.tile([P, sc, elem], g_dt, tag="G")
                    if "gathers" in ablate:
                        nc.vector.memset(G[:, :, 0:1], 1.0)
                    else:
                     g1 = nc.gpsimd.dma_gather(
                        out_ap=G[:, 0:n_lo, :], in_ap=tlo.ap(),
                        idxs_ap=ilo_t[:, s["lo_col0"]:s["lo_col0"] + n_lo * 8],
                        num_idxs=n_lo * P, num_idxs_reg=n_lo * P,
                        elem_size=elem, single_packet=False,
                        queue_num=nextq())
                     gathers.append(g1)
                     if n_hi:
                        g2 = nc.gpsimd.dma_gather(
                            out_ap=G[:, n_lo:sc, :], in_ap=thi.ap(),
                            idxs_ap=ihi_t[:, s["hi_col0"]:
                                          s["hi_col0"] + n_hi * 8],
                            num_idxs=n_hi * P, num_idxs_reg=n_hi * P,
                            elem_size=elem, single_packet=False,
                            queue_num=nextq())
                        gathers.append(g2)

                    # one-hot M over dst offsets for every chunk of the super
                    M = misc["pool2"].tile([P, sc, P], bf16, tag="M")
                    nc.vector.tensor_tensor(
                        out=M[:],
                        in0=doff_t[:, base:base + sc].unsqueeze(2)
                            .broadcast_to([P, sc, P]),
                        in1=iota_t[:].unsqueeze(1).broadcast_to([P, sc, P]),
                        op=OP.is_equal)

                    # ald replicated across partitions per tile:
                    # ald_rep[p, i, j] = ala[j, tiles[i]] via PE transpose +
                    # rank-1 outer product with a ones row
                    nt = len(s["tiles"])
                    ald_rep = pool.tile([P, nt, P], f32, tag="aldrep")
                    for i, t in enumerate(s["tiles"]):
                        rT0 = misc["ps_t"].tile([P, P], f32, tag="rT")
                        nc.tensor.transpose(out=rT0[0:1, :],
                                            in_=ala[:, t:t + 1],
                                            identity=ident_t[:])
                        row0 = pool.tile([1, P], f32, tag="alarow")
                        nc.vector.tensor_copy(row0[:], rT0[0:1, :])
                        rep_ps = misc["ps_t"].tile([P, P], f32, tag="rT")
                        nc.tensor.matmul(rep_ps[:], ones_row[:], row0[:],
                                         start=True, stop=True)
                        nc.vector.tensor_copy(ald_rep[:, i, :], rep_ps[:])

                    # ald per slot = one-hot dot: reduce_X(M * ald_rep)
                    e_t = pool.tile([P, sc], f32, tag="e")
                    c0 = 0
                    for h in (0, 1):
                        for i, t in enumerate(s["tiles"]):
                            nch = sum(1 for (tt, hh) in s["chunks"]
                                      if tt == t and hh == h)
                            if nch == 0:
                                continue
                            c1 = c0 + nch
                            mtmp = pool.tile([P, nch, P], bf16, tag="mtmp")
                            nc.vector.tensor_tensor(
                                out=mtmp[:], in0=M[:, c0:c1, :],
                                in1=ald_rep[:, i, :].unsqueeze(1)
                                    .broadcast_to([P, nch, P]),
                                op=OP.mult)
                            nc.vector.tensor_reduce(
                                e_t[:, c0:c1], mtmp[:],
                                axis=mybir.AxisListType.X, op=OP.add)
                            c0 = c1
                    # e = als[src] + ald[dst]; p = exp(leaky_relu(e))
                    nc.vector.tensor_tensor(out=e_t[:], in0=e_t[:],
                                            in1=G[:, :, als_c], op=OP.add)
                    e_s = pool.tile([P, sc], f32, tag="es")
                    nc.vector.tensor_scalar(out=e_s[:], in0=e_t[:],
                                            scalar1=NEG, scalar2=None,
                                            op0=OP.mult)
                    nc.vector.tensor_max(e_t[:], e_t[:], e_s[:])
                    p_t = pool.tile([P, sc], f32, tag="p")
                    nc.scalar.activation(p_t[:], e_t[:], AF.Exp)

                    U = {}
                    for t in s["tiles"]:
                        U[t] = psum_u.tile([P, row], f32, tag="U", name="U")
                    for b0 in range(0, sc, MP_BATCH):
                        nb = min(MP_BATCH, sc - b0)
                        Mp = misc["mp_pool"].tile([P, MP_BATCH, P], g_dt,
                                                  tag="Mp")
                        nc.vector.tensor_tensor(
                            out=Mp[:, 0:nb, :], in0=M[:, b0:b0 + nb, :],
                            in1=p_t[:, b0:b0 + nb].unsqueeze(2)
                                .broadcast_to([P, nb, P]),
                            op=OP.mult)
                        for i in range(nb):
                            gc = base + b0 + i
                            nc.tensor.matmul(
                                U[s["chunks"][b0 + i][0]][:], Mp[:, i, :],
                                G[:, b0 + i, 0:row],
                                start=meta["chunk_start"][gc],
                                stop=meta["chunk_stop"][gc])

                    for t in s["tiles"]:
                        s_inv = pool.tile([P, 1], f32, tag="sinv")
                        nc.vector.reciprocal(s_inv[:],
                                             U[t][:, ones_c:ones_c + 1])
                        if layer == 1:
                            relu1 = pool.tile([P, D_HID], f32, tag="relu1")
                            if with_bias1:
                                o1 = pool.tile([P, D_HID], f32, tag="o1")
                                nc.vector.tensor_scalar(
                                    out=o1[:], in0=U[t][:, 0:D_HID],
                                    scalar1=s_inv[:, 0:1], scalar2=None,
                                    op0=OP.mult)
                                nc.vector.tensor_add(o1[:], o1[:], b1_t[:])
                                nc.scalar.activation(relu1[:], o1[:], AF.Relu)
                            else:
                                nc.scalar.activation(
                                    relu1[:], U[t][:, 0:D_HID], AF.Relu,
                                    scale=s_inv[:, 0:1])
                            rT = misc["ps_t"].tile([P, P], f32, tag="rT")
                            nc.tensor.transpose(out=rT[:], in_=relu1[:],
                                                identity=ident_t[:])
                            rT_sb = pool.tile([P, P], f32, tag="rTsb")
                            nc.vector.tensor_copy(rT_sb[:], rT[:])
                            h2 = misc["ps_h"].tile([P, 18], f32, tag="h2")
                            nc.tensor.matmul(h2[:], rT_sb[:], w2_t[:],
                                             start=True, stop=True)
                            stg2 = pool.tile([P, ROW2], f32, tag="stg2")
                            nc.vector.tensor_copy(stg2[:, 0:18], h2[:])
                            nc.vector.memset(stg2[:, 18:19], 1.0)
                            side_writes.append(nc.sync.dma_start(
                                out=ag_in.ap()[t * P:(t + 1) * P, :],
                                in_=stg2[:]))
                        else:
                            o2 = pool.tile([P, D_OUT], f32, tag="o2")
                            nc.vector.tensor_scalar(
                                out=o2[:], in0=U[t][:, 0:D_OUT],
                                scalar1=s_inv[:, 0:1], scalar2=None,
                                op0=OP.mult)
                            if with_bias2:
                                nc.vector.tensor_add(o2[:], o2[:], b2_t[:])
                            B = pool.tile([P, N_GRAPHS], f32, tag="B")
                            nc.vector.tensor_tensor(
                                out=B[:],
                                in0=boff_t[:, t:t + 1]
                                    .broadcast_to([P, N_GRAPHS]),
                                in1=iota_t[:, 0:N_GRAPHS], op=OP.is_equal)
                            nc.tensor.matmul(misc["pool_ps"][:], B[:], o2[:],
                                             start=(t == 0),
                                             stop=(t == TILES_PER_CORE - 1))
                return gathers, ad_gathers, side_writes

            # ---------------- phase 2: layer 1 ----------------------------
            with (
                nc.named_scope("layer1"),
                tc.tile_pool(name="l1", bufs=G_BUFS) as pool,
                tc.tile_pool(name="l1b", bufs=2) as pool2,
                tc.tile_pool(name="l1a", bufs=1) as ala_pool,
                tc.tile_pool(name="l1mp", bufs=MP_BUFS) as mp_pool,
                tc.tile_pool(name="l1u", bufs=U1_BUFS, space="PSUM") as psum_u,
                tc.tile_pool(name="l1t", bufs=2, space="PSUM") as ps_t,
                tc.tile_pool(name="l1h", bufs=8 - 2 - U1_BUFS,
                             space="PSUM") as ps_h,
            ):
                for _r in range(rep["l1"]):
                    gathers1, adg1, ag_writes = aggregation_layer(
                        1, pool, psum_u,
                        dict(mp_pool=mp_pool, ps_t=ps_t, ps_h=ps_h,
                             pool2=pool2, ala=ala_pool))
                    for g in gathers1:
                        dep(g, fence0)
                    for g in adg1:
                        dep(g, fence1)

            # ---------------- phase 3: AllGather + expand ------------------
            scope3 = nc.enter_named_scope("allgather", False)
            if "cc" in ablate:
                cc1 = nc.sync.dma_start(out=ag_out.ap()[0:NODES_PER_CORE, :],
                                        in_=ag_in.ap())
            else:
                cc1 = nc.gpsimd.collective_compute(
                    "AllGather", OP.bypass, replica_groups=[core_ids],
                    ins=[ag_in[:]], outs=[ag_out[:]])
            dep(cc1, *ag_writes)
            ex1 = nc.sync.dma_start(out=t2lo.ap()[0:NLO, 0:ROW2],
                                    in_=ag_out.ap()[0:NLO, :])
            ex2 = nc.sync.dma_start(out=t2hi.ap()[0:NHI, 0:ROW2],
                                    in_=ag_out.ap()[NLO:NPAD, :])
            with nc.allow_non_contiguous_dma(
                    reason="4B/row ald column expand"):
                ex3 = nc.sync.dma_start(out=t2ad.ap()[:, 0:1],
                                        in_=ag_out.ap()[0:NAD, 17:18])
                ex4 = nc.sync.dma_start(out=t2ad.ap()[:, 1:2],
                                        in_=ag_out.ap()[NAD:NPAD, 17:18])
            dep(ex1, cc1)
            dep(ex2, cc1)
            dep(ex3, cc1)
            dep(ex4, cc1)
            fence2 = nc.sync.nop(nofuse=True, hint="fence_p3")
            dep(fence2, ex1, ex2)
            fence2b = nc.sync.nop(nofuse=True, hint="fence_p3b")
            dep(fence2b, ex3, ex4)
            nc.leave_named_scope("allgather", scope3[0], False)

            # ---------------- phase 4: layer 2 + pooling -------------------
            with (
                nc.named_scope("layer2"),
                tc.tile_pool(name="l2", bufs=G_BUFS) as pool,
                tc.tile_pool(name="l2b", bufs=2) as pool2,
                tc.tile_pool(name="l2a", bufs=1) as ala_pool,
                tc.tile_pool(name="l2mp", bufs=MP_BUFS) as mp_pool,
                tc.tile_pool(name="l2u", bufs=U2_BUFS, space="PSUM") as psum_u,
                tc.tile_pool(name="l2t", bufs=2, space="PSUM") as ps_t2,
                tc.tile_pool(name="poolps", bufs=1, space="PSUM") as pps,
            ):
                pool_ps = pps.tile([N_GRAPHS, D_OUT], f32)
                for _r in range(rep["l2"]):
                    gathers2, adg2, _ = aggregation_layer(
                        2, pool, psum_u,
                        dict(mp_pool=mp_pool, pool_ps=pool_ps, pool2=pool2,
                             ala=ala_pool, ps_t=ps_t2))
                    for g in gathers2:
                        dep(g, fence2)
                    for g in adg2:
                        dep(g, fence2b)
                    if _r == 0 and rep["l2"] > 1:
                        pool_ps2 = pps.tile([N_GRAPHS, D_OUT], f32,
                                            name="poolps2")
                        pool_ps = pool_ps2

                # -------------- phase 5: reduce + log_softmax --------------
                pp_sb = pool.tile([N_GRAPHS, D_OUT], f32)
                nc.vector.tensor_copy(pp_sb[:], pool_ps[:])
                w_ar = nc.sync.dma_start(out=ar_in.ap(), in_=pp_sb[:])
                if "cc" in ablate:
                    cc2 = nc.sync.dma_start(out=ar_out.ap(), in_=ar_in.ap())
                else:
                    cc2 = nc.gpsimd.collective_compute(
                        "AllReduce", OP.add, replica_groups=[core_ids],
                        ins=[ar_in[:]], outs=[ar_out[:]])
                dep(cc2, w_ar)
                red = pool.tile([N_GRAPHS, D_OUT], f32)
                r_ld = nc.sync.dma_start(out=red[:], in_=ar_out.ap())
                dep(r_ld, cc2)
                mean = pool.tile([N_GRAPHS, D_OUT], f32)
                nc.vector.tensor_scalar(out=mean[:], in0=red[:],
                                        scalar1=rcnt_t[:, 0:1], scalar2=None,
                                        op0=OP.mult)
                mx = pool.tile([N_GRAPHS, 1], f32)
                nc.vector.tensor_reduce(mx[:], mean[:],
                                        axis=mybir.AxisListType.X, op=OP.max)
                xm = pool.tile([N_GRAPHS, D_OUT], f32)
                nc.vector.tensor_scalar(out=xm[:], in0=mean[:],
                                        scalar1=mx[:, 0:1], scalar2=None,
                                        op0=OP.subtract)
                ex = pool.tile([N_GRAPHS, D_OUT], f32)
                nc.scalar.activation(ex[:], xm[:], AF.Exp)
                ssum = pool.tile([N_GRAPHS, 1], f32)
                nc.vector.tensor_reduce(ssum[:], ex[:],
                                        axis=mybir.AxisListType.X, op=OP.add)
                lse = pool.tile([N_GRAPHS, 1], f32)
                nc.scalar.activation(lse[:], ssum[:], AF.Ln)
                fin = pool.tile([N_GRAPHS, D_OUT], f32)
                nc.vector.tensor_scalar(out=fin[:], in0=xm[:],
                                        scalar1=lse[:, 0:1], scalar2=None,
                                        op0=OP.subtract)
                nc.sync.dma_start(out=out_d.ap(), in_=fin[:])

    nc.compile()
    return nc


# --------------------------------------------------------------- entry point
_CACHE = {}


def prepare(inputs):
    """Host preprocessing + (cached) program build. Returns (nc, in_maps)."""
    import ml_dtypes
    x = np.asarray(inputs["x"], np.float32)
    edge_index = np.asarray(inputs["edge_index"])
    batch = np.asarray(inputs["batch"])
    W1 = np.asarray(inputs["W1"], np.float32)
    a1s = np.asarray(inputs["a1_src"], np.float32)
    a1d = np.asarray(inputs["a1_dst"], np.float32)
    b1 = np.asarray(inputs["b1"], np.float32)
    W2 = np.asarray(inputs["W2"], np.float32)
    a2s = np.asarray(inputs["a2_src"], np.float32)
    a2d = np.asarray(inputs["a2_dst"], np.float32)
    b2 = np.asarray(inputs["b2"], np.float32)

    meta, per_core = preprocess(edge_index, batch)
    with_b1 = bool(np.abs(b1).max() > 0)
    with_b2 = bool(np.abs(b2).max() > 0)

    key = (meta["TC"], with_b1, with_b2,
           meta["idx_lo_cols"], meta["idx_hi_cols"],
           tuple(meta["chunk_start"]), tuple(meta["chunk_stop"]))
    if key not in _CACHE:
        _CACHE[key] = build_program(meta, with_b1, with_b2)
    nc = _CACHE[key]

    bf = ml_dtypes.bfloat16
    xT = np.zeros((P, NPAD), bf)
    xT[:, :N_NODES] = x.T.astype(bf)
    w1aug = np.concatenate([W1, (W1 @ a1s)[:, None], (W1 @ a1d)[:, None]],
                           axis=1).astype(bf)
    w2aug = np.concatenate([W2, (W2 @ a2s)[:, None], (W2 @ a2d)[:, None]],
                           axis=1).astype(np.float32)
    iota = np.tile(np.arange(P, dtype=np.int16), (P, 1))
    cnt = np.bincount(batch.astype(np.int64), minlength=N_GRAPHS)
    rcnt = (1.0 / np.maximum(cnt, 1)).astype(np.float32)[:, None]

    in_maps = []
    for k in range(NCORES):
        m = dict(xT=xT, w1aug=w1aug, w2aug=w2aug,
                 idx_lo=per_core[k]["idx_lo"], idx_hi=per_core[k]["idx_hi"],
                 dstoff=per_core[k]["dstoff"],
                 idx_adall=per_core[k]["idx_adall"],
                 admall=per_core[k]["admall"],
                 batchoff=per_core[k]["batchoff"],
                 iota=iota, recip_cnt=rcnt)
        if with_b1:
            m["b1b"] = np.tile(b1[None, :], (P, 1)).astype(np.float32)
        if with_b2:
            m["b2b"] = np.tile(b2[None, :], (P, 1)).astype(np.float32)
        in_maps.append(m)
    return nc, in_maps


def kernel(**inputs) -> np.ndarray:
    nc, in_maps = prepare(inputs)
    res = run_bass_kernel_spmd(nc, in_maps, list(range(NCORES)))
    return np.asarray(res.results[0]["out"], np.float32)


# revision 61
# speedup vs baseline: 1.1220x; 1.1220x over previous
"""Trainium2 Bass kernel for 2-layer GAT (nn_GAT_5970004541990).

Sharding: nodes padded 50000 -> 50176, sharded by destination across 8
NeuronCores (6272 nodes = 49 tiles of 128 each). Self-loops added. Host does
index preprocessing only; all FLOPs run on device.

v2 schedule: per dst tile, most edges sit in "dst-major" chunks where slot p
holds the r-th edge of dst node p, so the attention one-hot is the identity
and ald comes from a per-tile column (one 6272-row gather per layer instead
of one 256B row per edge). High-degree tails go to packed "overflow" chunks
that keep the one-hot path with a per-edge ald gather. Invalid dst-major
slots gather a dedicated all-zero poison row (contributes nothing).

Per core:
- Phase 1 (replicated): node table h_aug = x @ [W1 | W1 a1s | W1 a1d] + ones
  column in bf16, written to DRAM split at row 32640 (signed int16 gather
  indices); poison row appended to each split. ald column copied to a
  pair-packed side table.
- Phase 2 (layer 1): per super-tile of 2 dst tiles, dma_gather edge-chunk
  source rows; e = als[src] + ald[dst]; p = exp(leaky_relu(e)); per chunk a
  PE matmul U += Mp^T @ [feat|...|ones] accumulates numerator + denominator
  in PSUM (Mp = p-scaled identity for dst-major chunks, p-scaled one-hot for
  overflow chunks, built on the Activation engine); out1 = relu(U/s);
  layer-2 table shard h2_aug = out1 @ [W2 | W2 a2s | W2 a2d].
- Phase 3: AllGather compact layer-2 shards (50176 x 19 f32), expand into
  256B-row tables (+ poison rows).
- Phase 4 (layer 2): same aggregation, no relu; per-tile pooling matmul
  accumulates [64, 16] graph sums in PSUM.
- Phase 5: AllReduce pooled sums; mean via host 1/count; log_softmax.
All cores emit the identical [64, 16] output; kernel() returns core 0's.
"""
import sys

sys.path.insert(0, "/opt/trn_rl_repo")

import numpy as np

import concourse.bass as bass
import concourse.mybir as mybir
import concourse.tile as tile
from concourse.tile_rust import add_dep_helper
from concourse import bacc
from concourse.bass_utils import run_bass_kernel_spmd
from concourse.masks import make_identity

# ---------------------------------------------------------------- constants
N_NODES = 50000
N_GRAPHS = 64
D_HID = 128
D_OUT = 16
NEG = 0.2

P = 128
NCORES = 8
NPAD = 50176                     # 8 * 49 * 128
NODES_PER_CORE = NPAD // NCORES  # 6272
TILES_PER_CORE = NODES_PER_CORE // P  # 49
GLOBAL_TILES = NPAD // P         # 392
NLO = 32640                      # table split (signed int16 index limit,
NHI = NPAD - NLO                 # minus room for one poison row each)
LO_TILES = NLO // P              # 255
NAD = NPAD // 2                  # 25088 ald-table rows (pair-packed, < 32768)

S_TILES = 2                      # dst tiles per super-tile
Q_DM = 0.7                       # dst-major depth quantile
MP_BATCH = 4                     # chunks per batched Mp build on DVE
G_BUFS = 4                       # aggregation pool depth
U2_BUFS = 4                      # layer-2 U psum bufs
MP_BUFS = 8
XB_P1 = 8                        # phase-1 tiles per batched DMA
P1PS_BUFS = 4
U1_BUFS = 4
ELEM1B = 256                     # layer-1 table row bf16 (512B)
ELEM2 = 64                       # layer-2 table row f32 (256B)
ELEM_AD1 = 128                   # ald side table row bf16 (256B)
ROW1 = 131                       # meaningful cols: feat 0..127, als, ald, ones
ROW2 = 19                        # feat 0..15, als2, ald2, ones

f32 = mybir.dt.float32
i16 = mybir.dt.int16
bf16 = mybir.dt.bfloat16


def _ceil(a, b):
    return -(-a // b)


def _wrap_idx(flat):
    """dma_gather index layout: slot i reads wrapped[i%16, i//16]; the 16-row
    pattern is replicated to all 128 partitions (one per Q7 core)."""
    v16 = flat.astype(np.uint16).view(np.int16)
    return np.tile(v16.reshape(-1, 16).T, (8, 1))


def _pad_to(a, n, fill):
    out = np.full(n, fill, a.dtype)
    out[:len(a)] = a
    return out


# ------------------------------------------------------------ host indexing
def preprocess(edge_index, batch):
    src = np.concatenate(
        [edge_index[0].astype(np.int64), np.arange(NPAD, dtype=np.int64)])
    dst = np.concatenate(
        [edge_index[1].astype(np.int64), np.arange(NPAD, dtype=np.int64)])
    core = dst // NODES_PER_CORE
    edge_hi = src >= NLO

    # per (lo/hi, tile, node) degrees -> global K (dst-major depth) and OC
    # (overflow chunk count) per tile, identical on every core
    deg = np.zeros((2, NPAD), np.int64)
    np.add.at(deg[0], dst[~edge_hi], 1)
    np.add.at(deg[1], dst[edge_hi], 1)
    dg = deg.reshape(2, NCORES, TILES_PER_CORE, P)
    K = np.zeros((2, TILES_PER_CORE), np.int64)
    for t in range(TILES_PER_CORE):
        K[0, t] = int(np.quantile(dg[0, :, t, :], Q_DM))
        K[1, t] = int(np.quantile(dg[1, :, t, :], Q_DM))
    OC = np.zeros((2, TILES_PER_CORE), np.int64)
    for h in (0, 1):
        of = np.maximum(dg[h] - K[h][None, :, None], 0).sum(axis=2)
        OC[h] = _ceil(of, P).max(axis=0)

    # static chunk schedule (identical on every core); per super the lo
    # chunks come first so each of the two gathers covers one chunk range
    supers = []
    chunk_tiles, chunk_start, chunk_stop, chunk_of = [], [], [], []
    lo_c = hi_c = 0
    n_of_total = 0
    for s0 in range(0, TILES_PER_CORE, S_TILES):
        tiles = list(range(s0, min(s0 + S_TILES, TILES_PER_CORE)))
        of_base = n_of_total
        segs = []            # (kind, h, t, c0, c1, of0)
        chunks = []          # (t, of_rel)  of_rel = -1 for dst-major
        for h in (0, 1):
            for t in tiles:
                if K[h, t]:
                    segs.append(("dm", h, t, len(chunks),
                                 len(chunks) + K[h, t], -1))
                    chunks += [(t, -1)] * int(K[h, t])
                if OC[h, t]:
                    segs.append(("of", h, t, len(chunks),
                                 len(chunks) + OC[h, t], n_of_total))
                    chunks += [(t, n_of_total - of_base + i)
                               for i in range(int(OC[h, t]))]
                    n_of_total += int(OC[h, t])
        n_lo = sum(c1 - c0 for kind, h, t, c0, c1, o in segs if h == 0)
        n_hi = len(chunks) - n_lo
        first, last = {}, {}
        for ci, (t, _) in enumerate(chunks):
            first.setdefault(t, ci)
            last[t] = ci
        base = len(chunk_tiles)
        supers.append(dict(tiles=tiles, chunks=chunks, segs=segs,
                           n_lo=n_lo, n_hi=n_hi, base=base,
                           lo_col0=lo_c, hi_col0=hi_c,
                           of_base=of_base, n_of=n_of_total - of_base))
        lo_c += n_lo * 8
        hi_c += n_hi * 8
        for ci, (t, of_rel) in enumerate(chunks):
            chunk_tiles.append(t)
            chunk_start.append(ci == first[t])
            chunk_stop.append(ci == last[t])
            chunk_of.append(of_rel)
    TC = len(chunk_tiles)
    TC_OF = max(n_of_total, 1)
    of_ranges = [[] for _ in range(TILES_PER_CORE)]
    for s in supers:
        for kind, h, t, c0, c1, of0 in s["segs"]:
            if kind == "of":
                of_ranges[t].append((of0, c1 - c0))

    # per-core index tensors
    batch_pad = np.full(NPAD, 127, np.int64)
    batch_pad[:N_NODES] = batch.astype(np.int64)
    per_core = []
    for k in range(NCORES):
        m = core == k
        s_k = src[m]
        d_k = dst[m] - k * NODES_PER_CORE
        h_k = edge_hi[m]
        t_k = d_k // P
        o_k = d_k % P
        dm_idx, of_lists = {}, {}
        for t in range(TILES_PER_CORE):
            for h in (0, 1):
                mm = (t_k == t) & (h_k == (h == 1))
                s = s_k[mm] - (NLO if h else 0)
                o = o_k[mm]
                order = np.argsort(o, kind="stable")
                s, o = s[order], o[order]
                cnt = np.bincount(o, minlength=P)
                starts = np.concatenate([[0], np.cumsum(cnt)[:-1]])
                ranks = np.arange(len(o)) - starts[o]
                kk = int(K[h, t])
                dmm = ranks < kk
                poison = NLO if h == 0 else NHI
                tab = np.full((kk, P), poison, np.int64)
                tab[ranks[dmm], o[dmm]] = s[dmm]
                dm_idx[(h, t)] = tab
                of_lists[(h, t)] = (s[~dmm], o[~dmm])

        lo_cols, hi_cols = [], []
        doff_of = np.full((TC_OF, P), -1, np.int16)
        for s in supers:
            lo_flat, hi_flat = [], []
            for kind, h, t, c0, c1, of0 in s["segs"]:
                nch = c1 - c0
                if kind == "dm":
                    flat = dm_idx[(h, t)].reshape(-1)
                else:
                    srcs, offs = of_lists[(h, t)]
                    flat = _pad_to(srcs, nch * P, 0)
                    off2 = _pad_to(offs.astype(np.int16), nch * P,
                                   np.int16(-1)).reshape(nch, P)
                    doff_of[of0:of0 + nch] = off2
                (lo_flat if h == 0 else hi_flat).append(flat)
            lo_cols.append(_wrap_idx(np.concatenate(lo_flat)))
            if hi_flat:
                hi_cols.append(_wrap_idx(np.concatenate(hi_flat)))
        idx_lo = np.concatenate(lo_cols, axis=1)
        idx_hi = (np.concatenate(hi_cols, axis=1) if hi_cols
                  else np.zeros((P, 1), np.int16))
        gn = k * NODES_PER_CORE + np.arange(NODES_PER_CORE, dtype=np.int64)
        boff = batch_pad[k * NODES_PER_CORE:(k + 1) * NODES_PER_CORE]
        per_core.append(dict(
            idx_lo=np.ascontiguousarray(idx_lo),
            idx_hi=np.ascontiguousarray(idx_hi),
            doffof=np.ascontiguousarray(doff_of.T),
            idx_adall=np.ascontiguousarray(_wrap_idx(gn % NAD)),
            admall=np.ascontiguousarray(
                (gn >= NAD).astype(np.float32).reshape(TILES_PER_CORE, P).T),
            batchoff=np.ascontiguousarray(
                boff.reshape(TILES_PER_CORE, P).T.astype(np.int16))))

    meta = dict(supers=supers, TC=TC, TC_OF=TC_OF, of_ranges=of_ranges,
                chunk_start=chunk_start, chunk_stop=chunk_stop,
                chunk_of=chunk_of,
                idx_lo_cols=per_core[0]["idx_lo"].shape[1],
                idx_hi_cols=per_core[0]["idx_hi"].shape[1])
    return meta, per_core


# ------------------------------------------------------------- bass program
def build_program(meta, with_bias1, with_bias2, repeats=None, ablate=(),
                  nqueues=4):
    nc = bacc.Bacc("TRN2", target_bir_lowering=False, debug=False,
                   num_devices=NCORES, num_swdge_queues=nqueues)
    rep = dict(p1=1, l1=1, l2=1)
    rep.update(repeats or {})
    t1_dt = bf16
    elem1 = ELEM1B
    TC = meta["TC"]
    TC_OF = meta["TC_OF"]
    supers = meta["supers"]
    AF = mybir.ActivationFunctionType
    OP = mybir.AluOpType
    core_ids = list(range(NCORES))

    xT_d = nc.dram_tensor("xT", [P, NPAD], bf16, kind="ExternalInput")
    w1_d = nc.dram_tensor("w1aug", [P, 130], bf16, kind="ExternalInput")
    w2_d = nc.dram_tensor("w2aug", [P, 18], f32, kind="ExternalInput")
    ilo_d = nc.dram_tensor("idx_lo", [P, meta["idx_lo_cols"]], i16,
                           kind="ExternalInput")
    ihi_d = nc.dram_tensor("idx_hi", [P, meta["idx_hi_cols"]], i16,
                           kind="ExternalInput")
    doffof_d = nc.dram_tensor("doffof", [P, TC_OF], i16, kind="ExternalInput")
    iadall_d = nc.dram_tensor("idx_adall", [P, TILES_PER_CORE * 8], i16,
                              kind="ExternalInput")
    admall_d = nc.dram_tensor("admall", [P, TILES_PER_CORE], f32,
                              kind="ExternalInput")
    boff_d = nc.dram_tensor("batchoff", [P, TILES_PER_CORE], i16,
                            kind="ExternalInput")
    iota_d = nc.dram_tensor("iota", [P, P], i16, kind="ExternalInput")
    rcnt_d = nc.dram_tensor("recip_cnt", [N_GRAPHS, 1], f32,
                            kind="ExternalInput")
    if with_bias1:
        b1_d = nc.dram_tensor("b1b", [P, D_HID], f32, kind="ExternalInput")
    if with_bias2:
        b2_d = nc.dram_tensor("b2b", [P, D_OUT], f32, kind="ExternalInput")
    out_d = nc.dram_tensor("out", [N_GRAPHS, D_OUT], f32,
                           kind="ExternalOutput")

    # +1 row on each split: the all-zero poison row for invalid dm slots
    t1lo = nc.dram_tensor("t1lo", [NLO + 1, elem1], t1_dt)
    t1hi = nc.dram_tensor("t1hi", [NHI + 1, elem1], t1_dt)
    t1ad = nc.dram_tensor("t1ad", [NAD, ELEM_AD1], t1_dt)
    t2ad = nc.dram_tensor("t2ad", [NAD, ELEM2], f32)
    ag_in = nc.dram_tensor("ag_in", [NODES_PER_CORE, ROW2], f32)
    ag_out = nc.dram_tensor("ag_out", [NPAD, ROW2], f32, addr_space="Shared")
    t2lo = nc.dram_tensor("t2lo", [NLO + 1, ELEM2], f32)
    t2hi = nc.dram_tensor("t2hi", [NHI + 1, ELEM2], f32)
    ar_in = nc.dram_tensor("ar_in", [N_GRAPHS, D_OUT], f32)
    ar_out = nc.dram_tensor("ar_out", [N_GRAPHS, D_OUT], f32,
                            addr_space="Shared")

    def dep(after, *before):
        for b in before:
            add_dep_helper(after.ins, b.ins, reason="phase order")

    with tile.TileContext(nc) as tc:
        with tc.tile_pool(name="res", bufs=1) as res:
            iota_t = res.tile([P, P], i16)
            nc.sync.dma_start(out=iota_t[:], in_=iota_d.ap())
            doffof_t = res.tile([P, TC_OF], i16)
            nc.sync.dma_start(out=doffof_t[:], in_=doffof_d.ap())
            iadall_t = res.tile([P, TILES_PER_CORE * 8], i16)
            nc.sync.dma_start(out=iadall_t[:], in_=iadall_d.ap())
            admall_t = res.tile([P, TILES_PER_CORE], f32)
            nc.sync.dma_start(out=admall_t[:], in_=admall_d.ap())
            ilo_t = res.tile([P, meta["idx_lo_cols"]], i16)
            nc.sync.dma_start(out=ilo_t[:], in_=ilo_d.ap())
            ihi_t = res.tile([P, meta["idx_hi_cols"]], i16)
            nc.sync.dma_start(out=ihi_t[:], in_=ihi_d.ap())
            boff_t = res.tile([P, TILES_PER_CORE], i16)
            nc.sync.dma_start(out=boff_t[:], in_=boff_d.ap())
            w1_t = res.tile([P, 130], bf16)
            nc.sync.dma_start(out=w1_t[:], in_=w1_d.ap())
            w2_t = res.tile([P, 18], f32)
            nc.sync.dma_start(out=w2_t[:], in_=w2_d.ap())
            rcnt_t = res.tile([N_GRAPHS, 1], f32)
            nc.sync.dma_start(out=rcnt_t[:], in_=rcnt_d.ap())
            ident_t = res.tile([P, P], f32)
            make_identity(nc, ident_t[:])
            # replicated identity blocks for batched dst-major Mp builds
            ident_b4 = res.tile([P, MP_BATCH, P], bf16)
            ident_f4 = res.tile([P, MP_BATCH, P], f32)
            for i in range(MP_BATCH):
                make_identity(nc, ident_b4[:, i, :])
                make_identity(nc, ident_f4[:, i, :])
            ones_row = res.tile([1, P], f32)
            nc.vector.memset(ones_row[:], 1.0)
            z1 = res.tile([1, elem1], t1_dt)
            nc.vector.memset(z1[:], 0.0)
            z2 = res.tile([1, ELEM2], f32)
            nc.vector.memset(z2[:], 0.0)
            if with_bias1:
                b1_t = res.tile([P, D_HID], f32)
                nc.sync.dma_start(out=b1_t[:], in_=b1_d.ap())
            if with_bias2:
                b2_t = res.tile([P, D_OUT], f32)
                nc.sync.dma_start(out=b2_t[:], in_=b2_d.ap())

            # ---------------- phase 1: node table (replicated) -------------
            tab_writes = []
            lo_writes = []
            XB = XB_P1
            with (
                nc.named_scope("phase1_table"),
                tc.tile_pool(name="p1", bufs=3) as p1,
                tc.tile_pool(name="p1ps", bufs=P1PS_BUFS, space="PSUM") as p1ps,
            ):
              for _r in range(rep["p1"]):
                for gb in range(0, GLOBAL_TILES, XB):
                    nb = min(XB, GLOBAL_TILES - gb)
                    xt = p1.tile([P, XB * P], bf16, tag="xt")
                    nc.sync.dma_start(
                        out=xt[:, 0:nb * P],
                        in_=xT_d.ap()[:, gb * P:(gb + nb) * P])
                    stg = p1.tile([P, XB, ROW1], t1_dt, tag="stg")
                    for j in range(nb):
                        gt = gb + j
                        hps = p1ps.tile([P, 130], f32, tag="hps")
                        nc.tensor.matmul(hps[:], xt[:, j * P:(j + 1) * P],
                                         w1_t[:], start=True, stop=True)
                        nc.vector.tensor_copy(stg[:, j, 0:130], hps[:])
                    nc.vector.memset(stg[:, 0:nb, 130:131], 1.0)
                    # one DMA for nb tiles: DRAM rows (j*128+p) <-> src (p, j)
                    if gb + nb <= LO_TILES:
                        dst = t1lo.ap()[gb * P:(gb + nb) * P, 0:ROW1]
                        dst = dst.rearrange("(j p) c -> p j c", p=P)
                        w = nc.sync.dma_start(out=dst, in_=stg[:, 0:nb, :])
                        tab_writes.append(w)
                        lo_writes.append(w)
                    elif gb >= LO_TILES:
                        r0 = (gb - LO_TILES) * P
                        dst = t1hi.ap()[r0:r0 + nb * P, 0:ROW1]
                        dst = dst.rearrange("(j p) c -> p j c", p=P)
                        tab_writes.append(
                            nc.sync.dma_start(out=dst, in_=stg[:, 0:nb, :]))
                    else:
                        nlo_t = LO_TILES - gb
                        dst = t1lo.ap()[gb * P:NLO, 0:ROW1]
                        dst = dst.rearrange("(j p) c -> p j c", p=P)
                        w = nc.sync.dma_start(out=dst, in_=stg[:, 0:nlo_t, :])
                        tab_writes.append(w)
                        lo_writes.append(w)
                        dst = t1hi.ap()[0:(nb - nlo_t) * P, 0:ROW1]
                        dst = dst.rearrange("(j p) c -> p j c", p=P)
                        tab_writes.append(nc.sync.dma_start(
                            out=dst, in_=stg[:, nlo_t:nb, :]))
                # poison rows (all zeros)
                wlp = nc.sync.dma_start(
                    out=t1lo.ap()[NLO:NLO + 1, :], in_=z1[:])
                tab_writes.append(wlp)
                lo_writes.append(wlp)
                tab_writes.append(nc.sync.dma_start(
                    out=t1hi.ap()[NHI:NHI + 1, :], in_=z1[:]))
                # ald column table from the fat tables (3 strided copies,
                # ordered after every table write)
                fence_lo = nc.sync.nop(nofuse=True, hint="fence_p1lo")
                dep(fence_lo, *lo_writes)
                fence0 = nc.sync.nop(nofuse=True, hint="fence_p1w")
                dep(fence0, *tab_writes)
                with nc.allow_non_contiguous_dma(
                        reason="2B/row ald column build"):
                    c129 = 129
                    for ad_dma in (
                        nc.sync.dma_start(
                            out=t1ad.ap()[:, 0:1],
                            in_=t1lo.ap()[0:NAD, c129:c129 + 1]),
                        nc.sync.dma_start(
                            out=t1ad.ap()[0:NLO - NAD, 1:2],
                            in_=t1lo.ap()[NAD:NLO, c129:c129 + 1]),
                        nc.sync.dma_start(
                            out=t1ad.ap()[NLO - NAD:NAD, 1:2],
                            in_=t1hi.ap()[0:NHI, c129:c129 + 1]),
                    ):
                        dep(ad_dma, fence0)
                        tab_writes.append(ad_dma)

            fence1 = nc.sync.nop(nofuse=True, hint="fence_p1")
            dep(fence1, *tab_writes)   # tab_writes includes the 3 ad builds

            # --------------- shared aggregation loop ----------------------
            def aggregation_layer(layer, pool, psum_u, misc):
                """misc: dict with layer-specific psum pools / tiles."""
                if layer == 1:
                    elem, row, als_c, ones_c = elem1, ROW1, 128, 130
                    tlo, thi, tad = t1lo, t1hi, t1ad
                    g_dt = t1_dt
                    elem_ad, ad_dt = ELEM_AD1, t1_dt
                    ident_g = ident_b4
                else:
                    elem, row, als_c, ones_c = ELEM2, ROW2, 16, 18
                    tlo, thi, tad = t2lo, t2hi, t2ad
                    g_dt = f32
                    elem_ad, ad_dt = ELEM2, f32
                    ident_g = ident_f4
                gathers = []
                ad_gathers = []
                side_writes = []
                qctr = [0]

                def nextq():
                    q = qctr[0] % nqueues
                    qctr[0] += 1
                    return q

                # per-layer ald column for this core's own nodes:
                # ala[p, t] = ald[core*6272 + t*128 + p]
                ADA = misc["ala"].tile([P, TILES_PER_CORE, elem_ad], ad_dt,
                                       tag="ADA")
                if "gathers" in ablate:
                    nc.vector.memset(ADA[:, :, 0:2], 0.5)
                else:
                    ga = nc.gpsimd.dma_gather(
                        out_ap=ADA[:], in_ap=tad.ap(),
                        idxs_ap=iadall_t[:],
                        num_idxs=NODES_PER_CORE, num_idxs_reg=NODES_PER_CORE,
                        elem_size=elem_ad, single_packet=False,
                        queue_num=nextq())
                    ad_gathers.append(ga)
                ala = misc["ala"].tile([P, TILES_PER_CORE], f32, tag="ala")
                nc.vector.tensor_sub(ala[:], ADA[:, :, 1], ADA[:, :, 0])
                nc.vector.tensor_mul(ala[:], ala[:], admall_t[:])
                nc.vector.tensor_add(ala[:], ala[:], ADA[:, :, 0])

                # one one-hot build for every overflow chunk (covers all
                # supers); overflow ald comes from an on-chip one-hot dot
                # instead of a per-edge gather
                n_of_all = sum(s["n_of"] for s in supers)
                M = misc["ala"].tile([P, max(n_of_all, 1), P], bf16, tag="MOF")
                nc.vector.tensor_tensor(
                    out=M[:, 0:max(n_of_all, 1), :],
                    in0=doffof_t[:, 0:max(n_of_all, 1)]
                        .unsqueeze(2).broadcast_to([P, max(n_of_all, 1), P]),
                    in1=iota_t[:].unsqueeze(1)
                        .broadcast_to([P, max(n_of_all, 1), P]),
                    op=OP.is_equal)
                # ald_of[p, c] = ala[doffof[p, c], tile(c)]:
                # replicate ala column across partitions per tile (PE
                # transpose + rank-1 ones outer product), then reduce(M*rep)
                ald_of = misc["ala"].tile([P, max(n_of_all, 1)], f32,
                                          tag="aldof")
                for t in range(TILES_PER_CORE):
                    rngs = meta["of_ranges"][t]
                    if not rngs:
                        continue
                    rT0 = misc["ps_t"].tile([P, P], f32, tag="rT")
                    nc.tensor.transpose(out=rT0[0:1, :], in_=ala[:, t:t + 1],
                                        identity=ident_t[:])
                    row0 = misc["pool2"].tile([1, P], f32, tag="alarow")
                    nc.vector.tensor_copy(row0[:], rT0[0:1, :])
                    rep_ps = misc["ps_t"].tile([P, P], f32, tag="rT")
                    nc.tensor.matmul(rep_ps[:], ones_row[:], row0[:],
                                     start=True, stop=True)
                    rep_sb = misc["pool2"].tile([P, P], f32, tag="repsb")
                    nc.vector.tensor_copy(rep_sb[:], rep_ps[:])
                    for of0, ln in rngs:
                        mtmp = misc["pool2"].tile([P, ln, P], bf16,
                                                  tag="mtmp")
                        nc.vector.tensor_tensor(
                            out=mtmp[:], in0=M[:, of0:of0 + ln, :],
                            in1=rep_sb[:].unsqueeze(1)
                                .broadcast_to([P, ln, P]),
                            op=OP.mult)
                        nc.vector.tensor_reduce(
                            ald_of[:, of0:of0 + ln], mtmp[:],
                            axis=mybir.AxisListType.X, op=OP.add)

                for si, s in enumerate(supers):
                    sc = len(s["chunks"])
                    n_lo, n_hi, base = s["n_lo"], s["n_hi"], s["base"]
                    n_of, of_base = s["n_of"], s["of_base"]
                    G = pool.tile([P, sc, elem], g_dt, tag="G")
                    if "gathers" in ablate:
                        nc.vector.memset(G[:, :, 0:1], 1.0)
                    else:
                     g1 = nc.gpsimd.dma_gather(
                        out_ap=G[:, 0:n_lo, :], in_ap=tlo.ap(),
                        idxs_ap=ilo_t[:, s["lo_col0"]:s["lo_col0"] + n_lo * 8],
                        num_idxs=n_lo * P, num_idxs_reg=n_lo * P,
                        elem_size=elem, single_packet=False,
                        queue_num=nextq())
                     gathers.append(("lo", g1))
                     if n_hi:
                        g2 = nc.gpsimd.dma_gather(
                            out_ap=G[:, n_lo:sc, :], in_ap=thi.ap(),
                            idxs_ap=ihi_t[:, s["hi_col0"]:
                                          s["hi_col0"] + n_hi * 8],
                            num_idxs=n_hi * P, num_idxs_reg=n_hi * P,
                            elem_size=elem, single_packet=False,
                            queue_num=nextq())
                        gathers.append(("hi", g2))

                    # e = als[src] + ald[dst] per chunk segment
                    e_t = pool.tile([P, sc], f32, tag="e")
                    for kind, h, t, c0, c1, of0 in s["segs"]:
                        if kind == "dm":
                            nc.vector.tensor_scalar(
                                out=e_t[:, c0:c1], in0=G[:, c0:c1, als_c],
                                scalar1=ala[:, t:t + 1], scalar2=None,
                                op0=OP.add)
                        else:
                            r0, r1 = of0, of0 + (c1 - c0)
                            nc.vector.tensor_tensor(
                                out=e_t[:, c0:c1], in0=G[:, c0:c1, als_c],
                                in1=ald_of[:, r0:r1], op=OP.add)
                    e_s = pool.tile([P, sc], f32, tag="es")
                    nc.vector.tensor_scalar(out=e_s[:], in0=e_t[:],
                                            scalar1=NEG, scalar2=None,
                                            op0=OP.mult)
                    nc.vector.tensor_max(e_t[:], e_t[:], e_s[:])
                    p_t = pool.tile([P, sc], f32, tag="p")
                    nc.scalar.activation(p_t[:], e_t[:], AF.Exp)

                    U = {}
                    for t in s["tiles"]:
                        U[t] = psum_u.tile([P, row], f32, tag="U", name="U")
                    # batched Mp = p-scaled identity (dm) / one-hot (of),
                    # built MP_BATCH chunks at a time within each segment
                    for kind, h, t, c0, c1, of0 in s["segs"]:
                        for b0 in range(c0, c1, MP_BATCH):
                            nb = min(MP_BATCH, c1 - b0)
                            Mp = misc["mp_pool"].tile([P, MP_BATCH, P], g_dt,
                                                      tag="Mp")
                            if kind == "dm":
                                src_m = ident_g[:, 0:nb, :]
                            else:
                                r0 = of0 + (b0 - c0)
                                src_m = M[:, r0:r0 + nb, :]
                            nc.vector.tensor_tensor(
                                out=Mp[:, 0:nb, :], in0=src_m,
                                in1=p_t[:, b0:b0 + nb].unsqueeze(2)
                                    .broadcast_to([P, nb, P]),
                                op=OP.mult)
                            for i in range(nb):
                                gc = base + b0 + i
                                nc.tensor.matmul(
                                    U[t][:], Mp[:, i, :],
                                    G[:, b0 + i, 0:row],
                                    start=meta["chunk_start"][gc],
                                    stop=meta["chunk_stop"][gc])

                    for t in s["tiles"]:
                        s_inv = pool.tile([P, 1], f32, tag="sinv")
                        nc.vector.reciprocal(s_inv[:],
                                             U[t][:, ones_c:ones_c + 1])
                        if layer == 1:
                            relu1 = pool.tile([P, D_HID], f32, tag="relu1")
                            if with_bias1:
                                o1 = pool.tile([P, D_HID], f32, tag="o1")
                                nc.vector.tensor_scalar(
                                    out=o1[:], in0=U[t][:, 0:D_HID],
                                    scalar1=s_inv[:, 0:1], scalar2=None,
                                    op0=OP.mult)
                                nc.vector.tensor_add(o1[:], o1[:], b1_t[:])
                                nc.scalar.activation(relu1[:], o1[:], AF.Relu)
                            else:
                                nc.scalar.activation(
                                    relu1[:], U[t][:, 0:D_HID], AF.Relu,
                                    scale=s_inv[:, 0:1])
                            rT = misc["ps_t"].tile([P, P], f32, tag="rT")
                            nc.tensor.transpose(out=rT[:], in_=relu1[:],
                                                identity=ident_t[:])
                            rT_sb = pool.tile([P, P], f32, tag="rTsb")
                            nc.vector.tensor_copy(rT_sb[:], rT[:])
                            h2 = misc["ps_h"].tile([P, 18], f32, tag="h2")
                            nc.tensor.matmul(h2[:], rT_sb[:], w2_t[:],
                                             start=True, stop=True)
                            stg2 = pool.tile([P, ROW2], f32, tag="stg2")
                            nc.vector.tensor_copy(stg2[:, 0:18], h2[:])
                            nc.vector.memset(stg2[:, 18:19], 1.0)
                            side_writes.append(nc.sync.dma_start(
                                out=ag_in.ap()[t * P:(t + 1) * P, :],
                                in_=stg2[:]))
                        else:
                            o2 = pool.tile([P, D_OUT], f32, tag="o2")
                            nc.vector.tensor_scalar(
                                out=o2[:], in0=U[t][:, 0:D_OUT],
                                scalar1=s_inv[:, 0:1], scalar2=None,
                                op0=OP.mult)
                            if with_bias2:
                                nc.vector.tensor_add(o2[:], o2[:], b2_t[:])
                            B = pool.tile([P, N_GRAPHS], f32, tag="B")
                            nc.vector.tensor_tensor(
                                out=B[:],
                                in0=boff_t[:, t:t + 1]
                                    .broadcast_to([P, N_GRAPHS]),
                                in1=iota_t[:, 0:N_GRAPHS], op=OP.is_equal)
                            nc.tensor.matmul(misc["pool_ps"][:], B[:], o2[:],
                                             start=(t == 0),
                                             stop=(t == TILES_PER_CORE - 1))
                return gathers, ad_gathers, side_writes

            # ---------------- phase 2: layer 1 ----------------------------
            with (
                nc.named_scope("layer1"),
                tc.tile_pool(name="l1", bufs=G_BUFS) as pool,
                tc.tile_pool(name="l1b", bufs=2) as pool2,
                tc.tile_pool(name="l1a", bufs=1) as ala_pool,
                tc.tile_pool(name="l1mp", bufs=MP_BUFS) as mp_pool,
                tc.tile_pool(name="l1u", bufs=U1_BUFS, space="PSUM") as psum_u,
                tc.tile_pool(name="l1t", bufs=2, space="PSUM") as ps_t,
                tc.tile_pool(name="l1h", bufs=8 - 2 - U1_BUFS,
                             space="PSUM") as ps_h,
            ):
                for _r in range(rep["l1"]):
                    gathers1, adg1, ag_writes = aggregation_layer(
                        1, pool, psum_u,
                        dict(mp_pool=mp_pool, ps_t=ps_t, ps_h=ps_h,
                             pool2=pool2, ala=ala_pool))
                    for kind, g in gathers1:
                        dep(g, fence_lo if kind == "lo" else fence0)
                    for g in adg1:
                        dep(g, fence1)

            # ---------------- phase 3: AllGather + expand ------------------
            scope3 = nc.enter_named_scope("allgather", False)
            if "cc" in ablate:
                cc1 = nc.sync.dma_start(out=ag_out.ap()[0:NODES_PER_CORE, :],
                                        in_=ag_in.ap())
            else:
                cc1 = nc.gpsimd.collective_compute(
                    "AllGather", OP.bypass, replica_groups=[core_ids],
                    ins=[ag_in[:]], outs=[ag_out[:]])
            dep(cc1, *ag_writes)
            ex1 = nc.sync.dma_start(out=t2lo.ap()[0:NLO, 0:ROW2],
                                    in_=ag_out.ap()[0:NLO, :])
            ex2 = nc.sync.dma_start(out=t2hi.ap()[0:NHI, 0:ROW2],
                                    in_=ag_out.ap()[NLO:NPAD, :])
            exp1 = nc.sync.dma_start(out=t2lo.ap()[NLO:NLO + 1, :], in_=z2[:])
            exp2 = nc.sync.dma_start(out=t2hi.ap()[NHI:NHI + 1, :], in_=z2[:])
            with nc.allow_non_contiguous_dma(
                    reason="4B/row ald column expand"):
                ex3 = nc.sync.dma_start(out=t2ad.ap()[:, 0:1],
                                        in_=ag_out.ap()[0:NAD, 17:18])
                ex4 = nc.sync.dma_start(out=t2ad.ap()[:, 1:2],
                                        in_=ag_out.ap()[NAD:NPAD, 17:18])
            dep(ex1, cc1)
            dep(ex2, cc1)
            dep(ex3, cc1)
            dep(ex4, cc1)
            fence2 = nc.sync.nop(nofuse=True, hint="fence_p3")
            dep(fence2, ex1, ex2, exp1, exp2)
            fence2b = nc.sync.nop(nofuse=True, hint="fence_p3b")
            dep(fence2b, ex3, ex4)
            nc.leave_named_scope("allgather", scope3[0], False)

            # ---------------- phase 4: layer 2 + pooling -------------------
            with (
                nc.named_scope("layer2"),
                tc.tile_pool(name="l2", bufs=G_BUFS) as pool,
                tc.tile_pool(name="l2b", bufs=2) as pool2,
                tc.tile_pool(name="l2a", bufs=1) as ala_pool,
                tc.tile_pool(name="l2mp", bufs=MP_BUFS) as mp_pool,
                tc.tile_pool(name="l2u", bufs=U2_BUFS, space="PSUM") as psum_u,
                tc.tile_pool(name="l2t", bufs=2, space="PSUM") as ps_t2,
                tc.tile_pool(name="poolps", bufs=1, space="PSUM") as pps,
            ):
                pool_ps = pps.tile([N_GRAPHS, D_OUT], f32)
                for _r in range(rep["l2"]):
                    gathers2, adg2, _ = aggregation_layer(
                        2, pool, psum_u,
                        dict(mp_pool=mp_pool, pool_ps=pool_ps, pool2=pool2,
                             ala=ala_pool, ps_t=ps_t2))
                    for kind, g in gathers2:
                        dep(g, fence2)
                    for g in adg2:
                        dep(g, fence2b)
                    if _r == 0 and rep["l2"] > 1:
                        pool_ps2 = pps.tile([N_GRAPHS, D_OUT], f32,
                                            name="poolps2")
                        pool_ps = pool_ps2

                # -------------- phase 5: reduce + log_softmax --------------
                pp_sb = pool.tile([N_GRAPHS, D_OUT], f32)
                nc.vector.tensor_copy(pp_sb[:], pool_ps[:])
                w_ar = nc.sync.dma_start(out=ar_in.ap(), in_=pp_sb[:])
                if "cc" in ablate:
                    cc2 = nc.sync.dma_start(out=ar_out.ap(), in_=ar_in.ap())
                else:
                    cc2 = nc.gpsimd.collective_compute(
                        "AllReduce", OP.add, replica_groups=[core_ids],
                        ins=[ar_in[:]], outs=[ar_out[:]])
                dep(cc2, w_ar)
                red = pool.tile([N_GRAPHS, D_OUT], f32)
                r_ld = nc.sync.dma_start(out=red[:], in_=ar_out.ap())
                dep(r_ld, cc2)
                mean = pool.tile([N_GRAPHS, D_OUT], f32)
                nc.vector.tensor_scalar(out=mean[:], in0=red[:],
                                        scalar1=rcnt_t[:, 0:1], scalar2=None,
                                        op0=OP.mult)
                mx = pool.tile([N_GRAPHS, 1], f32)
                nc.vector.tensor_reduce(mx[:], mean[:],
                                        axis=mybir.AxisListType.X, op=OP.max)
                xm = pool.tile([N_GRAPHS, D_OUT], f32)
                nc.vector.tensor_scalar(out=xm[:], in0=mean[:],
                                        scalar1=mx[:, 0:1], scalar2=None,
                                        op0=OP.subtract)
                ex = pool.tile([N_GRAPHS, D_OUT], f32)
                nc.scalar.activation(ex[:], xm[:], AF.Exp)
                ssum = pool.tile([N_GRAPHS, 1], f32)
                nc.vector.tensor_reduce(ssum[:], ex[:],
                                        axis=mybir.AxisListType.X, op=OP.add)
                lse = pool.tile([N_GRAPHS, 1], f32)
                nc.scalar.activation(lse[:], ssum[:], AF.Ln)
                fin = pool.tile([N_GRAPHS, D_OUT], f32)
                nc.vector.tensor_scalar(out=fin[:], in0=xm[:],
                                        scalar1=lse[:, 0:1], scalar2=None,
                                        op0=OP.subtract)
                nc.sync.dma_start(out=out_d.ap(), in_=fin[:])

    nc.compile()
    return nc


# --------------------------------------------------------------- entry point
_CACHE = {}


def prepare(inputs):
    """Host preprocessing + (cached) program build. Returns (nc, in_maps)."""
    import ml_dtypes
    x = np.asarray(inputs["x"], np.float32)
    edge_index = np.asarray(inputs["edge_index"])
    batch = np.asarray(inputs["batch"])
    W1 = np.asarray(inputs["W1"], np.float32)
    a1s = np.asarray(inputs["a1_src"], np.float32)
    a1d = np.asarray(inputs["a1_dst"], np.float32)
    b1 = np.asarray(inputs["b1"], np.float32)
    W2 = np.asarray(inputs["W2"], np.float32)
    a2s = np.asarray(inputs["a2_src"], np.float32)
    a2d = np.asarray(inputs["a2_dst"], np.float32)
    b2 = np.asarray(inputs["b2"], np.float32)

    meta, per_core = preprocess(edge_index, batch)
    with_b1 = bool(np.abs(b1).max() > 0)
    with_b2 = bool(np.abs(b2).max() > 0)

    key = (meta["TC"], meta["TC_OF"], with_b1, with_b2,
           meta["idx_lo_cols"], meta["idx_hi_cols"],
           tuple(meta["chunk_start"]), tuple(meta["chunk_stop"]))
    if key not in _CACHE:
        _CACHE[key] = build_program(meta, with_b1, with_b2)
    nc = _CACHE[key]

    bf = ml_dtypes.bfloat16
    xT = np.zeros((P, NPAD), bf)
    xT[:, :N_NODES] = x.T.astype(bf)
    w1aug = np.concatenate([W1, (W1 @ a1s)[:, None], (W1 @ a1d)[:, None]],
                           axis=1).astype(bf)
    w2aug = np.concatenate([W2, (W2 @ a2s)[:, None], (W2 @ a2d)[:, None]],
                           axis=1).astype(np.float32)
    iota = np.tile(np.arange(P, dtype=np.int16), (P, 1))
    cnt = np.bincount(batch.astype(np.int64), minlength=N_GRAPHS)
    rcnt = (1.0 / np.maximum(cnt, 1)).astype(np.float32)[:, None]

    in_maps = []
    for k in range(NCORES):
        m = dict(xT=xT, w1aug=w1aug, w2aug=w2aug,
                 idx_lo=per_core[k]["idx_lo"], idx_hi=per_core[k]["idx_hi"],
                 doffof=per_core[k]["doffof"],
                 idx_adall=per_core[k]["idx_adall"],
                 admall=per_core[k]["admall"],
                 batchoff=per_core[k]["batchoff"],
                 iota=iota, recip_cnt=rcnt)
        if with_b1:
            m["b1b"] = np.tile(b1[None, :], (P, 1)).astype(np.float32)
        if with_b2:
            m["b2b"] = np.tile(b2[None, :], (P, 1)).astype(np.float32)
        in_maps.append(m)
    return nc, in_maps


def kernel(**inputs) -> np.ndarray:
    nc, in_maps = prepare(inputs)
    res = run_bass_kernel_spmd(nc, in_maps, list(range(NCORES)))
    return np.asarray(res.results[0]["out"], np.float32)
